# revision 14
# baseline (speedup 1.0000x reference)
"""GCN (3-layer + global_add_pool + linear head) on 8 Trainium2 NeuronCores.

Strategy
--------
- Nodes (and their incident edges, grouped by destination) are sharded across
  the 8 cores: core c owns destination nodes [c*SHARD, (c+1)*SHARD).
- gcn_norm's edge coefficients (norm = dinv[row]*w*dinv[col]) depend only on
  the graph structure + edge weights; they are precomputed on the host and
  baked into per-edge one-hot scatter matrices S (bf16).
- Per layer:
    1. Each core transforms ITS OWN node shard: h = in_fm.T @ W.T via the
       TensorEngine (input kept feature-major so no transposes are needed),
       casts to bf16 and writes a node-major [SHARD, 64] bounce buffer.
    2. AllGather assembles the full [N, 64] bf16 node-feature table in every
       core's HBM (concat along dim 0 == global node id order).
    3. Each core aggregates its destination shard: per 128-edge chunk,
       an indirect DMA gathers the 64-wide source rows (one row per SBUF
       partition) and a matmul with the host-built one-hot S chunk
       [128 edges x TILE_D dsts] scatter-adds into a PSUM tile
       [64 feat x TILE_D dsts] (feature-major), accumulating over chunks.
       Bias+ReLU are fused into the ScalarEngine PSUM evacuation.
- Pooling: per node-tile, PE-transpose the aggregated features to node-major
  and matmul with a host-built batch one-hot [128 nodes x 128 graphs],
  accumulating hG in PSUM; AllReduce combines partial graph sums.
- Head: logits^T = Wl @ hG^T on PE, bias via ACT, PE-transpose, then
  log_softmax along the free dim with DVE/ACT ops.

The compiled program is identical on all 8 cores (SPMD); only the input data
(per-core edge slots / S matrices / indices / x shard) differs. The per-tile
chunk schedule is made uniform across cores by padding each (core, tile) edge
list to the max chunk count over cores for that tile (pad slots gather row 0
and carry S-weight 0, so they contribute nothing).
"""

import math

import ml_dtypes
import numpy as np

# ----------------------------------------------------------------- constants
N_NODES = 100000
N_EDGES = 1000000
D = 64
N_CLASSES = 10
N_GRAPHS = 128
N_CORES = 8

TILE_D = 128      # destination nodes per PSUM aggregation tile
GROUP = 7         # agg tiles per gather/S-stream group
STAGE_GROUP = 4   # transform tiles batched per bounce DMA

_cache = {}


# ------------------------------------------------------------- preprocessing
class Plan:
    """Host-side graph preprocessing: slot layout, S matrices, indices."""

    def __init__(self, edge_index, edge_weight, batch, x, n_nodes=N_NODES,
                 n_cores=N_CORES, tile_d=TILE_D, n_graphs=N_GRAPHS):
        n = n_nodes
        shard = n // n_cores
        self.n, self.n_cores, self.shard, self.tile_d = n, n_cores, shard, tile_d
        self.n_graphs = n_graphs
        nt = math.ceil(shard / tile_d)
        self.nt = nt

        ei = np.asarray(edge_index).astype(np.int64)
        ew = np.asarray(edge_weight).astype(np.float64)
        bat = np.asarray(batch).astype(np.int64)

        loop = np.arange(n, dtype=np.int64)
        row = np.concatenate([ei[0], loop])
        col = np.concatenate([ei[1], loop])
        w = np.concatenate([ew, np.ones(n)])
        deg = np.bincount(col, weights=w, minlength=n)
        dinv = np.where(deg > 0, 1.0 / np.sqrt(deg), 0.0)
        norm = (dinv[row] * w * dinv[col]).astype(np.float32)

        core = col // shard
        dloc = col - core * shard
        tl = dloc // tile_d
        key = (core * nt + tl).astype(np.int64)
        counts = np.bincount(key, minlength=n_cores * nt).reshape(n_cores, nt)
        # chunks per tile: uniform across cores (max), >= 1
        cpt = np.maximum((counts + 127) // 128, 1).max(axis=0)
        self.cpt = cpt.astype(np.int64)
        slots_per_tile = self.cpt * 128
        tile_slot_off = np.concatenate([[0], np.cumsum(slots_per_tile)])
        self.tile_ch_off = (tile_slot_off // 128).astype(np.int64)
        total_slots = int(tile_slot_off[-1])
        self.total_ch = total_slots // 128

        order = np.argsort(key, kind="stable")
        sk = key[order]
        starts = np.searchsorted(sk, np.arange(n_cores * nt))
        rank = np.arange(len(order)) - starts[sk]
        e_core = sk // nt
        e_tile = sk % nt
        slotpos = tile_slot_off[e_tile] + rank

        srcs = np.zeros((n_cores, total_slots), np.int32)
        norms = np.zeros((n_cores, total_slots), np.float32)
        dls = np.zeros((n_cores, total_slots), np.int32)
        srcs[e_core, slotpos] = row[order].astype(np.int32)
        norms[e_core, slotpos] = norm[order]
        dls[e_core, slotpos] = (dloc[order] - e_tile * tile_d).astype(np.int32)
        self.srcs, self.norms, self.dls = srcs, norms, dls

        # device idx layout [128, total_ch]: idx[p, k] = srcs[k*128 + p]
        self.idx_dev = np.ascontiguousarray(
            srcs.reshape(n_cores, self.total_ch, 128).transpose(0, 2, 1))

        # S (bf16) layout [128, total_ch*tile_d]
        pp = np.arange(total_slots) % 128
        kk = np.arange(total_slots) // 128
        s_dev = np.zeros((n_cores, 128, self.total_ch, tile_d), np.float32)
        for c in range(n_cores):
            s_dev[c][pp, kk, dls[c]] = norms[c]
        self.s_dev = np.ascontiguousarray(
            s_dev.reshape(n_cores, 128, self.total_ch * tile_d)
        ).astype(ml_dtypes.bfloat16)

        # pooling one-hot per core [shard, n_graphs] f32
        bs = bat.reshape(n_cores, shard)
        self.pool_oh = (bs[:, :, None] ==
                        np.arange(n_graphs)[None, None, :]).astype(np.float32)

        # x shards, feature-major [64, shard] f32
        xs = np.asarray(x).astype(np.float32)
        self.xT = np.ascontiguousarray(
            xs.reshape(n_cores, shard, -1).transpose(0, 2, 1))

        # schedule helpers for the builder
        self.tile_sizes = [min(tile_d, shard - t * tile_d) for t in range(nt)]
        self.n_groups = math.ceil(nt / GROUP)
        self.chg_max = max(
            int(self.tile_ch_off[min((g + 1) * GROUP, nt)] -
                self.tile_ch_off[g * GROUP])
            for g in range(self.n_groups))


# ------------------------------------------------------------ numpy emulator
def emulate(plan: Plan, inputs, stages=None):
    """Numpy re-implementation of the exact device pipeline (incl. bf16
    rounding of tables and S) for fast numeric validation."""
    bf = ml_dtypes.bfloat16
    x = np.asarray(inputs["x"], np.float32)
    W = [np.asarray(inputs[k], np.float32) for k in ("W1", "W2", "W3")]
    b = [np.asarray(inputs[k], np.float32) for k in ("b1", "b2", "b3")]
    Wl = np.asarray(inputs["Wl"], np.float32)
    bl = np.asarray(inputs["bl"], np.float32)
    n, shard, tile_d = plan.n, plan.shard, plan.tile_d

    s_f32 = plan.s_dev.astype(np.float32)  # [cores, 128, total_ch*tile_d]
    feat = x
    for li in range(3):
        table = (feat @ W[li].T).astype(bf).astype(np.float32)  # [n, 64]
        if stages is not None:
            stages[f"table{li}"] = table
        agg = np.zeros((n, D), np.float32)
        for c in range(plan.n_cores):
            msg = table[plan.idx_dev[c].T.reshape(-1)]  # [slots, 64] slot order
            for t in range(plan.nt):
                c0, c1 = plan.tile_ch_off[t], plan.tile_ch_off[t + 1]
                acc = np.zeros((D, tile_d), np.float32)
                for k in range(c0, c1):
                    m = msg[k * 128:(k + 1) * 128]           # [128, 64]
                    s = s_f32[c][:, k * tile_d:(k + 1) * tile_d]
                    acc += m.T @ s
                nd = plan.tile_sizes[t]
                base = c * shard + t * tile_d
                agg[base:base + nd] += acc[:, :nd].T
        feat = agg + b[li][None, :]
        if li < 2:
            feat = np.maximum(feat, 0.0)
        if stages is not None:
            stages[f"feat{li}"] = feat
    bat = np.asarray(inputs["batch"]).astype(np.int64)
    hg = np.zeros((plan.n_graphs, D), np.float32)
    np.add.at(hg, bat, feat)
    logits = hg @ Wl.T + bl[None, :]
    m = logits.max(axis=1, keepdims=True)
    t = logits - m
    lp = t - np.log(np.exp(t).sum(axis=1, keepdims=True))
    return hg, lp


# ------------------------------------------------------------ device program
def build_program(plan: Plan, debug=False):
    import concourse.bacc as bacc
    import concourse.bass as bass
    import concourse.mybir as mybir
    import concourse.tile as tile
    from concourse.masks import make_identity

    f32 = mybir.dt.float32
    bf16 = mybir.dt.bfloat16
    i32 = mybir.dt.int32
    AF = mybir.ActivationFunctionType
    ALU = mybir.AluOpType

    n, shard, tile_d, nt = plan.n, plan.shard, plan.tile_d, plan.nt
    total_ch = plan.total_ch
    n_cores = plan.n_cores
    G = plan.n_graphs
    rg = [list(range(n_cores))]

    nc = bacc.Bacc("TRN2", target_bir_lowering=False, debug=False,
                   num_devices=n_cores)

    # ---- I/O
    xT_d = nc.dram_tensor("xT", [D, shard], f32, kind="ExternalInput").ap()
    idx_d = nc.dram_tensor("idx", [128, total_ch], i32, kind="ExternalInput").ap()
    s_d = nc.dram_tensor("S", [128, total_ch * tile_d], bf16,
                         kind="ExternalInput").ap()
    oh_d = nc.dram_tensor("poolOH", [shard, G], f32, kind="ExternalInput").ap()
    w_d = [nc.dram_tensor(f"W{l}T", [D, D], f32, kind="ExternalInput").ap()
           for l in (1, 2, 3)]
    b_d = [nc.dram_tensor(f"b{l}", [D, 1], f32, kind="ExternalInput").ap()
           for l in (1, 2, 3)]
    wl_d = nc.dram_tensor("WlT", [D, N_CLASSES], f32, kind="ExternalInput").ap()
    bl_d = nc.dram_tensor("bl", [N_CLASSES, 1], f32, kind="ExternalInput").ap()
    hg_out = nc.dram_tensor("hg_out", [G, D], f32, kind="ExternalOutput").ap()
    lp_out = nc.dram_tensor("lp_out", [G, N_CLASSES], f32,
                            kind="ExternalOutput").ap()
    if debug:
        dbg_t = [nc.dram_tensor(f"dbg_t{l}", [n, D], bf16,
                                kind="ExternalOutput").ap() for l in range(3)]
        dbg_r = [nc.dram_tensor(f"dbg_r{l}", [D, shard], f32,
                                kind="ExternalOutput").ap() for l in range(3)]
        dbg_hg = nc.dram_tensor("dbg_hg", [G, D], f32,
                                kind="ExternalOutput").ap()

    with tile.TileContext(nc) as tc:
        with (
            tc.tile_pool(name="const", bufs=1) as constp,
            tc.tile_pool(name="rbuf", bufs=1) as rbufp,
            tc.tile_pool(name="msgp", bufs=2) as msgp,
            tc.tile_pool(name="sp", bufs=2) as sp,
            tc.tile_pool(name="xtp", bufs=3) as xtp,
            tc.tile_pool(name="stgp", bufs=3) as stgp,
            tc.tile_pool(name="ohp", bufs=3) as ohp,
            tc.tile_pool(name="headp", bufs=1) as headp,
            tc.tile_pool(name="ps_agg", bufs=3, space="PSUM") as ps_agg,
            tc.tile_pool(name="ps_tr", bufs=2, space="PSUM") as ps_tr,
            tc.tile_pool(name="ps_misc", bufs=2, space="PSUM") as ps_misc,
            tc.tile_pool(name="ps_hg", bufs=1, space="PSUM") as ps_hg,
            tc.tile_pool(name="dram", bufs=1, space="DRAM") as dram,
        ):
            # ---- constants into SBUF
            idx_sb = constp.tile([128, total_ch], i32, name="idx_sb")
            nc.sync.dma_start(idx_sb[:], idx_d[:])
            w_sb, b_sb = [], []
            for l in range(3):
                wt = constp.tile([D, D], f32, name=f"w{l}_sb", tag=f"w{l}_sb")
                nc.sync.dma_start(wt[:], w_d[l][:])
                w_sb.append(wt)
                bt = constp.tile([D, 1], f32, name=f"bias{l}_sb",
                                 tag=f"bias{l}_sb")
                nc.sync.dma_start(bt[:], b_d[l][:])
                b_sb.append(bt)
            wl_sb = constp.tile([D, N_CLASSES], f32, name="wl_sb")
            nc.sync.dma_start(wl_sb[:], wl_d[:])
            bl_sb = constp.tile([N_CLASSES, 1], f32, name="bl_sb")
            nc.sync.dma_start(bl_sb[:], bl_d[:])
            ident = constp.tile([128, 128], f32, name="ident")
            make_identity(nc, ident[:])

            # ---- DRAM scratch
            tables = [dram.tile([n, D], bf16, name=f"table{l}", tag=f"table{l}",
                                addr_space="Shared") for l in range(3)]
            bounces = [dram.tile([shard, D], bf16, name=f"bounce{l}",
                                 tag=f"bounce{l}") for l in range(3)]
            hg_ar_in = dram.tile([G, D], f32, name="hg_ar_in")
            hg_ar_out = dram.tile([G, D], f32, name="hg_ar_out",
                                  addr_space="Shared")

            # R buffers: feature-major activations of own shard
            r_tiles = []
            for l in range(3):
                r = rbufp.tile([D, shard], f32, name=f"r{l}", tag="R", bufs=2)
                r_tiles.append(r)

            def transform(src_of_tile, wt, bounce, table):
                """h_own = src.T @ W.T, cast bf16, write bounce, AllGather."""
                j = 0
                while j < nt:
                    jn = min(STAGE_GROUP, nt - j)
                    full = all(plan.tile_sizes[j + q] == 128 for q in range(jn))
                    stg = stgp.tile([128, STAGE_GROUP * D], bf16, name="stg",
                                    tag="stg")
                    for q in range(jn):
                        t = j + q
                        njd = plan.tile_sizes[t]
                        pt = ps_tr.tile([128, D], f32, name="pt", tag="pt")
                        nc.tensor.matmul(pt[:njd, :], src_of_tile(t), wt[:],
                                         start=True, stop=True)
                        nc.vector.tensor_copy(
                            stg[:njd, q * D:(q + 1) * D], pt[:njd, :])
                    r0 = j * tile_d
                    rows = sum(plan.tile_sizes[j + q] for q in range(jn))
                    if full:
                        nc.sync.dma_start(
                            bounce[r0:r0 + rows, :].rearrange(
                                "(k p) d -> p k d", p=128),
                            stg[:, :jn * D].rearrange("p (k d) -> p k d", d=D))
                    else:
                        for q in range(jn):
                            t = j + q
                            njd = plan.tile_sizes[t]
                            nc.sync.dma_start(
                                bounce[t * tile_d:t * tile_d + njd, :],
                                stg[:njd, q * D:(q + 1) * D])
                    j += jn
                nc.gpsimd.collective_compute(
                    "AllGather", ALU.bypass, replica_groups=rg,
                    ins=[bounce.opt()], outs=[table.opt()])

            def aggregate(table, bias, relu, r_out):
                """Scatter-add aggregation of own dst shard from `table`.

                The bedrock runtime only supports indirect DMA at one
                descriptor per partition, so each 128-edge chunk is its own
                gather call (out [128, 64], idx [128, 1])."""
                for g in range(plan.n_groups):
                    t0 = g * GROUP
                    t1 = min(t0 + GROUP, nt)
                    c0 = int(plan.tile_ch_off[t0])
                    c1 = int(plan.tile_ch_off[t1])
                    chg = c1 - c0
                    s_sb = sp.tile([128, plan.chg_max * tile_d], bf16,
                                   name="s_sb", tag="s_sb")
                    nc.sync.dma_start(s_sb[:, :chg * tile_d],
                                      s_d[:, c0 * tile_d:c1 * tile_d])
                    for t in range(t0, t1):
                        ps = ps_agg.tile([D, tile_d], f32, name="ps", tag="ps")
                        k0 = int(plan.tile_ch_off[t]) - c0
                        kn = int(plan.cpt[t])
                        for k in range(kn):
                            kk = k0 + k
                            msg = msgp.tile([128, D], bf16, name="msg",
                                            tag="msg")
                            nc.gpsimd.indirect_dma_start(
                                out=msg[:, :],
                                out_offset=None,
                                in_=table[:, :],
                                in_offset=bass.IndirectOffsetOnAxis(
                                    ap=idx_sb[:, c0 + kk:c0 + kk + 1], axis=0),
                            )
                            nc.tensor.matmul(
                                ps[:, :],
                                msg[:, :],
                                s_sb[:, kk * tile_d:(kk + 1) * tile_d],
                                start=(k == 0), stop=(k == kn - 1))
                        nd = plan.tile_sizes[t]
                        nc.scalar.activation(
                            r_out[:, t * tile_d:t * tile_d + nd],
                            ps[:, :nd],
                            AF.Relu if relu else AF.Identity,
                            bias=bias[:])

            # ---------------- layer 1..3
            def xt_src(t):
                njd = plan.tile_sizes[t]
                xt = xtp.tile([D, tile_d], f32, name="xt", tag="xt")
                nc.sync.dma_start(xt[:, :njd],
                                  xT_d[:, t * tile_d:t * tile_d + njd])
                return xt[:, :njd]

            transform(xt_src, w_sb[0], bounces[0], tables[0])
            aggregate(tables[0], b_sb[0], True, r_tiles[0])

            def r0_src(t):
                njd = plan.tile_sizes[t]
                return r_tiles[0][:, t * tile_d:t * tile_d + njd]

            transform(r0_src, w_sb[1], bounces[1], tables[1])
            aggregate(tables[1], b_sb[1], True, r_tiles[1])

            def r1_src(t):
                njd = plan.tile_sizes[t]
                return r_tiles[1][:, t * tile_d:t * tile_d + njd]

            transform(r1_src, w_sb[2], bounces[2], tables[2])
            aggregate(tables[2], b_sb[2], False, r_tiles[2])

            if debug:
                for l in range(3):
                    nc.sync.dma_start(dbg_t[l][:], tables[l][:])
                    nc.sync.dma_start(dbg_r[l][:], r_tiles[l][:])

            # ---------------- pooling: hG[g, f] = sum_nodes onehot^T @ A3
            hg_ps = ps_hg.tile([G, D], f32, name="hg_ps")
            for t in range(nt):
                njd = plan.tile_sizes[t]
                tp = ps_misc.tile([128, D], f32, name="tp", tag="pmisc")
                nc.tensor.transpose(tp[:njd, :],
                                    r_tiles[2][:, t * tile_d:t * tile_d + njd],
                                    ident[:D, :D])
                a3 = stgp.tile([128, D], f32, name="a3", tag="a3")
                nc.vector.tensor_copy(a3[:njd, :], tp[:njd, :])
                oh = ohp.tile([128, G], f32, name="oh", tag="oh")
                nc.sync.dma_start(oh[:njd, :],
                                  oh_d[t * tile_d:t * tile_d + njd, :])
                nc.tensor.matmul(hg_ps[:, :], oh[:njd, :], a3[:njd, :],
                                 start=(t == 0), stop=(t == nt - 1))
            hg_sb = headp.tile([G, D], f32, name="hg_sb")
            nc.vector.tensor_copy(hg_sb[:], hg_ps[:])
            if debug:
                nc.sync.dma_start(dbg_hg[:], hg_sb[:])
            nc.sync.dma_start(hg_ar_in[:], hg_sb[:])
            nc.gpsimd.collective_compute(
                "AllReduce", ALU.add, replica_groups=rg,
                ins=[hg_ar_in.opt()], outs=[hg_ar_out.opt()])

            hg2 = headp.tile([G, D], f32, name="hg2")
            nc.sync.dma_start(hg2[:], hg_ar_out[:])
            nc.sync.dma_start(hg_out[:], hg2[:])

            # ---------------- head
            hgT_ps = ps_misc.tile([D, G], f32, name="hgT_ps", tag="pmisc")
            nc.tensor.transpose(hgT_ps[:], hg2[:], ident[:])
            hgT = headp.tile([D, G], f32, name="hgT")
            nc.vector.tensor_copy(hgT[:], hgT_ps[:])
            lgT_ps = ps_misc.tile([N_CLASSES, G], f32, name="lgT_ps",
                                  tag="pmisc")
            nc.tensor.matmul(lgT_ps[:], wl_sb[:], hgT[:], start=True, stop=True)
            lgT = headp.tile([N_CLASSES, G], f32, name="lgT")
            nc.scalar.activation(lgT[:], lgT_ps[:], AF.Identity, bias=bl_sb[:])
            lg_ps = ps_misc.tile([G, N_CLASSES], f32, name="lg_ps", tag="pmisc")
            nc.tensor.transpose(lg_ps[:], lgT[:], ident[:N_CLASSES, :N_CLASSES])
            lg = headp.tile([G, N_CLASSES], f32, name="lg")
            nc.vector.tensor_copy(lg[:], lg_ps[:])
            mx = headp.tile([G, 1], f32, name="mx")
            nc.vector.reduce_max(out=mx[:], in_=lg[:],
                                 axis=mybir.AxisListType.X)
            t2 = headp.tile([G, N_CLASSES], f32, name="t2")
            nc.vector.tensor_scalar(t2[:], lg[:], mx[:], None, ALU.subtract)
            ex = headp.tile([G, N_CLASSES], f32, name="ex")
            nc.scalar.activation(ex[:], t2[:], AF.Exp)
            sm = headp.tile([G, 1], f32, name="sm")
            nc.vector.reduce_sum(out=sm[:], in_=ex[:],
                                 axis=mybir.AxisListType.X)
            lns = headp.tile([G, 1], f32, name="lns")
            nc.scalar.activation(lns[:], sm[:], AF.Ln)
            lp = headp.tile([G, N_CLASSES], f32, name="lp")
            nc.vector.tensor_scalar(lp[:], t2[:], lns[:], None, ALU.subtract)
            nc.sync.dma_start(lp_out[:], lp[:])

    nc.compile()
    return nc


# ------------------------------------------------------------------- driver
def _in_maps(plan: Plan, inputs):
    w1 = np.ascontiguousarray(np.asarray(inputs["W1"], np.float32).T)
    w2 = np.ascontiguousarray(np.asarray(inputs["W2"], np.float32).T)
    w3 = np.ascontiguousarray(np.asarray(inputs["W3"], np.float32).T)
    wl = np.ascontiguousarray(np.asarray(inputs["Wl"], np.float32).T)
    b1 = np.asarray(inputs["b1"], np.float32).reshape(D, 1)
    b2 = np.asarray(inputs["b2"], np.float32).reshape(D, 1)
    b3 = np.asarray(inputs["b3"], np.float32).reshape(D, 1)
    bl = np.asarray(inputs["bl"], np.float32).reshape(N_CLASSES, 1)
    maps = []
    for c in range(plan.n_cores):
        maps.append({
            "xT": plan.xT[c],
            "idx": plan.idx_dev[c],
            "S": plan.s_dev[c],
            "poolOH": plan.pool_oh[c],
            "W1T": w1, "W2T": w2, "W3T": w3,
            "b1": b1, "b2": b2, "b3": b3,
            "WlT": wl, "bl": bl,
        })
    return maps


def _get_compiled(inputs):
    key = "prog"
    if key not in _cache:
        plan = Plan(inputs["edge_index"], inputs["edge_weight"],
                    inputs["batch"], inputs["x"])
        nc = build_program(plan)
        _cache[key] = (plan, nc)
    return _cache[key]


def run(inputs, trace=False):
    from concourse.bass_utils import run_bass_kernel_spmd
    plan, nc = _get_compiled(inputs)
    maps = _in_maps(plan, inputs)
    res = run_bass_kernel_spmd(nc, maps, core_ids=list(range(plan.n_cores)),
                               trace=trace)
    r0 = res.results[0]
    return (r0["hg_out"], r0["lp_out"]), res


def kernel(**inputs):
    (hg, lp), _ = run(inputs, trace=False)
    return hg, lp


# ------------------------------------------------------------------ benching
def bench(inputs, iters=10):
    """Warm device-execution wall time (ns) of the SPMD program on 8 cores.

    Mirrors bass2jax.run_bass_via_pjrt's multi-core path but keeps inputs
    device-resident so repeated calls time only dispatch + NEFF execution.
    """
    import time as _time

    import jax
    from jax.experimental.shard_map import shard_map
    from jax.sharding import Mesh, NamedSharding, PartitionSpec

    import concourse.bass2jax as b2j
    import concourse.mybir as mybir

    plan, nc = _get_compiled(inputs)
    maps = _in_maps(plan, inputs)
    n_cores = plan.n_cores

    b2j.install_neuronx_cc_hook()
    partition_name = (nc.partition_id_tensor.name
                      if nc.partition_id_tensor else None)
    in_names, out_names, out_avals, zero_outs = [], [], [], []
    for alloc in nc.m.functions[0].allocations:
        if not isinstance(alloc, mybir.MemoryLocationSet):
            continue
        name = alloc.memorylocations[0].name
        if alloc.kind == "ExternalInput":
            if name == partition_name:
                continue
            in_names.append(name)
        elif alloc.kind == "ExternalOutput":
            out_names.append(name)
            shape = tuple(alloc.tensor_shape)
            dtype = mybir.dt.np(alloc.dtype)
            out_avals.append(jax.core.ShapedArray(shape, dtype))
            zero_outs.append(np.zeros(shape, dtype))
    n_params = len(in_names)
    in_names = in_names + out_names

    if partition_name is not None:
        in_names.append(partition_name)

    def _body(*args):
        operands = list(args)
        if partition_name is not None:
            operands.append(b2j.partition_id_tensor())
        outs = b2j._bass_exec_p.bind(
            *operands,
            out_avals=tuple(out_avals),
            in_names=tuple(in_names),
            out_names=tuple(out_names),
            lowering_input_output_aliases=(),
            sim_require_finite=True,
            sim_require_nnan=True,
            nc=nc,
        )
        return tuple(outs)

    devices = jax.devices()[:n_cores]
    mesh = Mesh(np.asarray(devices), ("core",))
    in_specs = (PartitionSpec("core"),) * (n_params + len(out_names))
    out_specs = (PartitionSpec("core"),) * len(out_names)
    sharded = jax.jit(shard_map(_body, mesh=mesh, in_specs=in_specs,
                                out_specs=out_specs, check_rep=False),
                      keep_unused=True)
    sh = NamedSharding(mesh, PartitionSpec("core"))
    dev_in = [
        jax.device_put(
            np.concatenate([np.asarray(maps[c][nm]) for c in range(n_cores)],
                           axis=0), sh)
        for nm in in_names[:n_params]
    ]
    dev_zero = [
        jax.device_put(np.zeros((n_cores * z.shape[0], *z.shape[1:]), z.dtype),
                       sh) for z in zero_outs
    ]
    # warmup (includes compile on first ever call)
    out = sharded(*dev_in, *dev_zero)
    jax.block_until_ready(out)
    times = []
    for _ in range(iters):
        t0 = _time.perf_counter()
        out = sharded(*dev_in, *dev_zero)
        jax.block_until_ready(out)
        times.append(_time.perf_counter() - t0)
    return float(np.median(times) * 1e9)
